# revision 40
# baseline (speedup 1.0000x reference)
"""Trainium2 Bass kernel: causal GQA attention (prefill), 8-core tensor-parallel.

Problem: q [4096, 16*128], k/v [4096, 4*128], f32. 16 query heads, 4 kv heads,
head_dim 128, causal softmax(q k^T / sqrt(d)) v.

Sharding: head-parallel across 8 NeuronCores. Core c owns query heads
{2c, 2c+1}, which both belong to kv head c//2. Each core runs full causal
attention over its 2 heads; no cross-core communication.

Per-core kernel (N=4096 tokens, 32 token tiles of 128):
  - Inputs DMA'd in chunks into f32 SBUF staging. q/k tiles are cast to bf16
    (q on GpSimd, k on VectorE with the score pre-scale folded in) and
    transposed by the DMA xbar (dma_start_transpose, 4 tiles per dispatch,
    blocked 3D destination) into qT/kT [d=128, 4096] bf16 -- no TensorE or
    PSUM involvement in transposes at all. kT carries 128*log2(e)/sqrt(d)
    so scores arrive as u = 128*log2(exp(s/sqrt(d))). v is cast to bf16
    with a ones-column appended (vones) so the PV matmul's column 128
    accumulates the softmax denominator for free.
  - Scores computed transposed: S^T[m, qcols] = kT_j.T @ qT (PSUM f32) over
    [128, <=1024] strips (2 PSUM banks x 3 buffers).
  - Softmax exp is split across TWO engines: ScalarE activation
    (exp(u*ln2/128)) for most strips, and a custom Vector-engine DVE op
    (EXP2_SPLICE_ANT, registered at import) for every third strip: it
    computes the bf16 BIT PATTERN of 2^(u/128) directly -- magic-constant
    round to the 128-grid, |r| + r^2 correction polynomial spliced onto the
    exponent field, written as int16 and bitcast to bf16. Masked scores
    (-1e9) saturate the int16 convert to 0x8000 = -0.0. ~0.26% rms error on
    those strips, comparable to bf16 quantization.
  - pT strips are directly the stationary operand for PV:
    acc[q, 129] += pT_j.T @ [v_j | 1].
  - PE stream is software-pipelined with a TWO-strip skew and block-level
    interleave: strip i's PV matmuls (129 cols, shorter than their
    LDWEIGHTS) are emitted between strip i+2's QK matmuls (256+ cols) so
    every PV weight load hides under a QK matmul and no PV ever stalls the
    in-order PE queue waiting on exp.
  - Query groups are 2 tiles (256 cols); both accumulators share ONE PSUM
    bank (two banks, double buffered, so a group's PVs never wait on the
    previous group's normalize), zero-filled once per group by a lazy dummy
    matmul so all PV matmuls accumulate with start=False.
  - Causal: only blocks j<=i computed. The two diagonal blocks of a group
    are packed as one 512-col unit at strip offset 0, masked by a SINGLE
    start=True PE matmul (maskT.T @ [I|0|I|0]) writing [mask|0|mask|0];
    scores accumulate on top (start=False). No max-subtraction (scores ~
    N(0,1): exp cannot overflow).
  - Normalize: one batched reciprocal per group ([128, GQ]), then
    out[q, d] = acc[:, :128] * rec on VectorE, DMA to DRAM.
"""

import sys

for _p in ("/opt/trn_rl_repo",):
    if _p not in sys.path:
        sys.path.insert(0, _p)

import numpy as np

import concourse.bacc as bacc
import concourse.mybir as mybir
import concourse.tile as tile
from concourse.bass_utils import run_bass_kernel_spmd
from concourse.masks import make_identity

F32 = mybir.dt.float32
BF16 = mybir.dt.bfloat16
I16 = mybir.dt.int16

N = 4096
D = 128
H_PER_CORE = 2
NCORES = 8
NT = N // 128          # 32 token tiles
GQ = 2                 # q-tiles per group (256 query columns)
NG = NT // GQ          # 16 groups
SCALE = float(1.0 / np.sqrt(np.float32(D)))
MASK_VAL = -1e9
STRIP_W = 1024         # strip width in score cols (2 PSUM banks)
DIAG_W = 512           # padded diagonal unit width (1 PSUM bank)
SKEW = 2               # strips of PE software-pipeline skew

# scores arrive pre-scaled: u = s * PRESCALE with exp(s*SCALE) = 2^(u/128)
PRESCALE = float(np.float32(128.0 * np.log2(np.e) * SCALE))
ACT_SCALE = float(np.log(2.0) / 128.0)  # ScalarE: exp(u*ln2/128) = 2^(u/128)

# EXP2_SPLICE_ANT constants (see _register_exp2_op)
EXP2_M = float(np.float32(1.5 * 2**30))  # magic: snaps u to the 128 grid
EXP2_A0 = 16256.0                        # 127*128: bf16 exponent bias splice
EXP2_G = -0.3448068249203487             # |r| coefficient (kink term)
EXP2_C = 0.0027100904874714006           # r^2 coefficient
DVE_MOD = 3                              # strips with idx % MOD == PHASE -> DVE
DVE_PHASE = 1


def _exp2_ref(in0, in1, s0, s1, imm2):
    x = in0.astype(np.float32)
    t = (x + np.float32(s0)).astype(np.float32)
    n = (t - np.float32(s0)).astype(np.float32)
    ar = np.abs((x - n).astype(np.float32))
    return ((np.float32(imm2) * ar + in1) * ar + (x + np.float32(s1))).astype(
        np.float32
    )


def _register_exp2_op():
    """Register the custom DVE op computing int16 bf16-bits of 2^(Src0/128).

    out = ((C2*|r| + C3)*|r|) + (Src0 + C1),  r = Src0 - round128(Src0)
    with C0 = 1.5*2^30 (magic), C1 = 16256, C2 = r^2 coeff, C3 = |r| coeff
    (spilled to Src1 as a [P,1] broadcast). Output dtype int16: the f32->i16
    convert rounds and saturates, so masked scores (~-1e9) land at -0.0."""
    import concourse.dve_ops as dve_ops_mod
    from concourse.dve_ops import DveOp
    from concourse.dve_spec import (
        AluOp,
        Bin,
        C0,
        C1,
        C2,
        C3,
        Spec,
        Src0,
        _spill_c3_to_src1,
    )

    name = "EXP2_SPLICE_ANT"
    for o in dve_ops_mod.OPS:
        if o.name == name:
            return o
    dve_ops_mod._SUB_OPCODE_FOR_NAME[name] = 17

    t = Src0 + C0
    n = t - C0
    ar = Bin(AluOp.ABSOLUTE_DIFF, Src0, n)
    v1 = Src0 + C1
    u1 = ar * C2
    u2 = u1 + C3
    u3 = u2 * ar
    body = u3 + v1
    op = DveOp(
        name,
        Spec(body=_spill_c3_to_src1(body), reference=_exp2_ref),
        subdim=False,
        uops_sha={"v3": "f8e4d417c7e21f04", "v4": "78ec334198c67423"},
    )
    dve_ops_mod.OPS.append(op)
    return op


def _build():
    exp2_op = _register_exp2_op()
    nc = bacc.Bacc(
        "TRN2",
        target_bir_lowering=False,
        debug=False,
        enable_asserts=False,
        num_devices=NCORES,
    )
    q_d = nc.dram_tensor("q", [N, H_PER_CORE * D], F32, kind="ExternalInput").ap()
    k_d = nc.dram_tensor("k", [N, D], F32, kind="ExternalInput").ap()
    v_d = nc.dram_tensor("v", [N, D], F32, kind="ExternalInput").ap()
    o_d = nc.dram_tensor("out", [N, H_PER_CORE * D], F32, kind="ExternalOutput").ap()

    with tile.TileContext(nc) as tc:
        with (
            tc.tile_pool(name="consts", bufs=1) as consts,
            tc.tile_pool(name="big", bufs=1) as big,
            tc.tile_pool(name="cstage", bufs=4) as cstage,
            tc.tile_pool(name="pstage", bufs=8) as pstage,
            tc.tile_pool(name="outp", bufs=2) as outp,
            tc.tile_pool(name="rpool", bufs=8) as rpool,
            tc.tile_pool(name="pst", bufs=3, space="PSUM") as psum_st,
            tc.tile_pool(name="pacc", bufs=2, space="PSUM") as psum_acc,
        ):
            # diag mask, stored TRANSPOSED (maskT[q, m] = 0 if m <= q else
            # MASK_VAL) so a matmul maskT.T @ I writes mask[m, q] into PSUM;
            # the diagonal QK matmuls then accumulate scores on top.
            maskT = consts.tile([128, 128], BF16)
            nc.gpsimd.memset(maskT, 0.0)
            nc.gpsimd.affine_select(
                out=maskT,
                in_=maskT,
                compare_op=mybir.AluOpType.is_ge,
                fill=MASK_VAL,
                base=0,
                # keep 0 where (x=q) - (y=m) >= 0, i.e. m <= q
                pattern=[[-1, 128]],
                channel_multiplier=1,
            )
            zeros_bf = consts.tile([128, 128], BF16)
            nc.vector.memset(zeros_bf, 0.0)
            # [I | 0 | I | 0]: moving operand writing the whole 512-col
            # diagonal unit [mask | 0 | mask | 0] in ONE start=True matmul
            # (two start=True in the same bank would clear each other's
            # has_written bits).
            iext = consts.tile([128, DIAG_W], BF16)
            nc.vector.memset(iext, 0.0)
            make_identity(nc, iext[:, 0:128], nomemset=True)
            make_identity(nc, iext[:, 256:384], nomemset=True)
            # [P, 1] broadcast of the spilled |r| coefficient for the DVE op
            gconst = consts.tile([128, 1], F32)
            nc.vector.memset(gconst, EXP2_G)

            # f32 staging ([p, tile, col], p = token % 128). q is split per
            # head so h1's 2MB streams in during h0's compute instead of
            # competing with startup-critical h0/k/v loads.
            qst = [
                big.tile([128, NT, D], F32, tag=f"qst{h}", name=f"qst{h}")
                for h in range(H_PER_CORE)
            ]
            kst = big.tile([128, NT, D], F32, tag="kst")
            vst = big.tile([128, NT, D], F32, tag="vst")

            # [d, token-tile, token%128] -- 3D so DMA-transpose destinations
            # and matmul operands are natural tile slices
            qT = [
                big.tile([128, NT, 128], BF16, tag=f"qT{h}", name=f"qT{h}")
                for h in range(H_PER_CORE)
            ]
            kT = big.tile([128, NT, 128], BF16, tag="kT")
            vones = big.tile([128, NT, 129], BF16, tag="vones")

            # ---- input loads (4-tile chunks) ----
            # Loads, DMA-xbar transposes and output stores all share the
            # Sync queue and each dispatch costs ~0.5-0.7us of desc-gen, so
            # loads are interleaved with the prep stream rather than all
            # upfront (which would park the first transpose ~25us out).
            def dma_load_kv(t):
                nc.sync.dma_start(
                    out=kst[:, t : t + 4, :],
                    in_=k_d[t * 128 : (t + 4) * 128, :].rearrange(
                        "(t p) c -> p t c", p=128
                    ),
                )
                nc.sync.dma_start(
                    out=vst[:, t : t + 4, :],
                    in_=v_d[t * 128 : (t + 4) * 128, :].rearrange(
                        "(t p) c -> p t c", p=128
                    ),
                )

            def dma_load_q(h, t):
                nc.sync.dma_start(
                    out=qst[h][:, t : t + 4, :],
                    in_=q_d[
                        t * 128 : (t + 4) * 128, h * D : (h + 1) * D
                    ].rearrange("(t p) c -> p t c", p=128),
                )

            # Prep stages for one 4-tile batch. Casts park their tile in
            # `cst` until the (separately scheduled) transpose dispatch.
            cst = {}

            def prep_loads(t):
                dma_load_kv(t)
                dma_load_q(0, t)
                dma_load_q(1, t)

            def prep_casts(t, fast=False):
                ck = cstage.tile([128, 4, 128], BF16, tag="ck", name="ck")
                # fold the score pre-scale into the k cast; ScalarE keeps it
                # out of the DVE queue (whose exp strips feed PV tightly)
                nc.scalar.mul(ck, kst[:, t : t + 4, :], PRESCALE)
                cst[("k", t)] = ck
                for h in range(H_PER_CORE):
                    cq = cstage.tile(
                        [128, 4, 128], BF16, tag=f"cq{h}", name=f"cq{h}"
                    )
                    if fast:
                        # startup-critical: VectorE cast (~0.6us) instead of
                        # the slow GpSimd ucode (~1.9us)
                        nc.vector.tensor_copy(cq, qst[h][:, t : t + 4, :])
                    else:
                        nc.gpsimd.tensor_copy(cq, qst[h][:, t : t + 4, :])
                    cst[("q", h, t)] = cq
                nc.gpsimd.tensor_copy(vones[:, t : t + 4, 0:128], vst[:, t : t + 4, :])
                nc.gpsimd.memset(vones[:, t : t + 4, 128:129], 1.0)

            def prep_tps(t):
                # by the time these reach the queue head their casts are
                # long done, so they never stall the Sync queue
                nc.sync.dma_start_transpose(
                    out=kT[:, t : t + 4, :], in_=cst.pop(("k", t))[:, :, :]
                )
                for h in range(H_PER_CORE):
                    nc.sync.dma_start_transpose(
                        out=qT[h][:, t : t + 4, :],
                        in_=cst.pop(("q", h, t))[:, :, :],
                    )

            # upfront: minimize the bytes ahead of the first k/q0 transposes
            # (only k+q0 tiles 0..3 = 0.75MB), so the first QK can start
            # ~3us after the k chunk lands; q1/v and the rest stream after
            nc.sync.dma_start(
                out=kst[:, 0:4, :],
                in_=k_d[0:512, :].rearrange("(t p) c -> p t c", p=128),
            )
            dma_load_q(0, 0)
            ck0 = cstage.tile([128, 4, 128], BF16, tag="ck", name="ck")
            nc.scalar.mul(ck0, kst[:, 0:4, :], PRESCALE)
            cq00 = cstage.tile([128, 4, 128], BF16, tag="cq0", name="cq0")
            nc.vector.tensor_copy(cq00, qst[0][:, 0:4, :])
            nc.sync.dma_start_transpose(out=kT[:, 0:4, :], in_=ck0[:, :, :])
            nc.sync.dma_start_transpose(out=qT[0][:, 0:4, :], in_=cq00[:, :, :])
            dma_load_q(1, 0)
            nc.sync.dma_start(
                out=vst[:, 0:4, :],
                in_=v_d[0:512, :].rearrange("(t p) c -> p t c", p=128),
            )
            cq10 = cstage.tile([128, 4, 128], BF16, tag="cq1", name="cq1")
            nc.vector.tensor_copy(cq10, qst[1][:, 0:4, :])
            nc.sync.dma_start_transpose(out=qT[1][:, 0:4, :], in_=cq10[:, :, :])
            nc.gpsimd.tensor_copy(vones[:, 0:4, 0:128], vst[:, 0:4, :])
            nc.gpsimd.memset(vones[:, 0:4, 128:129], 1.0)
            prep_loads(4)
            prep_loads(8)
            prep_loads(12)

            # ---- build the full strip schedule ----
            # HEADS ARE INTERLEAVED (h0 g, h1 g, h0 g+1, ...): early groups
            # are tiny (1-2 strips) and burn through new k/q tiles faster
            # than the load+cast+transpose pipeline can deliver them; with
            # both heads consuming each tile the demand rate halves.
            # Strip 0 of each group leads with the 512-col diagonal unit
            # (blocks j=2g w256 at 0, j=2g+1 w128 at 256, pad 384..512)
            # followed by up to 2 off-diagonal blocks; remaining
            # off-diagonals pack 4 per strip. Every block start stays
            # 256-aligned inside 512-col PSUM banks.
            units = []  # flat list over groups/heads/strips
            for g in range(NG):
                for h in range(H_PER_CORE):
                    qc0 = g * GQ * 128
                    offd = [(j, qc0, GQ * 128) for j in range(g * GQ)]
                    strips = []
                    first_blocks = [
                        (g * GQ, qc0, 256, 0),
                        (g * GQ + 1, qc0 + 128, 128, 256),
                    ]
                    so = DIAG_W
                    while offd and so < STRIP_W:
                        j, c0, w = offd.pop(0)
                        first_blocks.append((j, c0, w, so))
                        so += w
                    strips.append((first_blocks, so, True))
                    while offd:
                        blocks = []
                        so = 0
                        while offd and so < STRIP_W:
                            j, c0, w = offd.pop(0)
                            blocks.append((j, c0, w, so))
                            so += w
                        strips.append((blocks, so, False))

                    # last-emitted PV per accumulator gets the stop flag
                    last_pv = {}
                    for si, (blocks, _, _) in enumerate(strips):
                        for j, c0, w, so_b in blocks:
                            for il in range(GQ):
                                if g * GQ + il >= j:
                                    last_pv[il] = (si, j)

                    ns = len(strips)
                    for si, (blocks, width, has_diag) in enumerate(strips):
                        units.append(
                            dict(
                                h=h,
                                g=g,
                                si=si,
                                idx=len(units),
                                blocks=blocks,
                                width=width,
                                has_diag=has_diag,
                                last=(si == ns - 1),
                                last_pv=last_pv,
                            )
                        )

            # ---- stage-scheduled prep/IO emission ----
            # Each prep stage (load -> cast -> transpose) is emitted a few
            # units after its upstream stage, so by the time an instruction
            # reaches the head of its in-order queue its dependencies are
            # complete and it never parks the queue. All batches land well
            # before first use (tiles 4t first read by group pair 2t).
            pg = {}
            for idx, u in enumerate(units):
                pg.setdefault(u["g"], idx)
            nu = len(units)
            sched = [[] for _ in range(nu + 1)]

            def at(idx, fn, *args):
                sched[min(max(idx, 0), nu)].append((fn, args))

            at(1, prep_casts, 4, True)
            at(3, prep_tps, 4)
            at(3, prep_casts, 8)
            at(5, prep_tps, 8)
            at(5, prep_casts, 12)
            at(7, prep_tps, 12)
            for t in range(16, NT, 4):
                gA = (t - 12) // 2
                i0 = pg[gA]
                at(i0, prep_loads, t)
                at(i0 + 2, prep_casts, t)
                at(i0 + 4, prep_tps, t)

            # ---- software-pipelined emission, two-strip skew ----
            # The PE queue is in-order: a strip's PV matmuls would stall the
            # queue waiting on that strip's exp. Interleave strip i's PV
            # matmuls BETWEEN strip i+2's QK matmuls: by then exp(i) is long
            # done, and each 129-col PV weight load hides under a 256-col QK
            # matmul. Group normalize rides in the PV stream.
            gstate = {}  # (h, g) -> dict(acc2, accs)
            obatch = {}  # h -> current 4-group output batch tile

            def pv_chunks(u):
                """Per-block PV emitters for strip u (plus group bookkeeping)."""
                chunks = []
                g0 = u["g"] * GQ
                key = (u["h"], u["g"])

                def emit_block(blk, first, last):
                    def f():
                        gs = gstate.get(key)
                        if gs is None and first:
                            # both q-tile accumulators in ONE psum bank; a
                            # single dummy matmul (zeros stationary,
                            # start=True) clears has_written for the whole
                            # bank so every PV accumulates with start=False.
                            acc2 = psum_acc.tile(
                                [128, GQ, 129], F32, tag="acc", name="acc2"
                            )
                            gs = gstate[key] = dict(
                                acc2=acc2, accs=[acc2[:, a, :] for a in range(GQ)]
                            )
                            nc.tensor.matmul(
                                acc2[:, 0, 0:1],
                                lhsT=zeros_bf,
                                rhs=iext[:, 128:129],
                                start=True,
                                stop=True,
                            )
                        gs = gstate[key]
                        j, c0, w, so_b = blk
                        pt2 = u["pt2"]
                        for il in range(GQ):
                            i = g0 + il
                            if i < j:
                                continue
                            off = so_b + i * 128 - c0
                            nc.tensor.matmul(
                                gs["accs"][il],
                                lhsT=pt2[:, off : off + 128],
                                rhs=vones[:, j, :],
                                start=False,
                                stop=(u["last_pv"][il] == (u["si"], j)),
                            )
                        if last and u["last"]:
                            gsf = gstate.pop(key)
                            rec = rpool.tile(
                                [128, GQ, 1], F32, tag="rec", name="rec"
                            )
                            nc.vector.reciprocal(rec, gsf["acc2"][:, :, 128:129])
                            # normalized outputs collect in an 8-tile batch
                            # buffer; ONE output DMA per 4 groups (each
                            # dispatch costs ~0.5us of Sync desc-gen)
                            gg = u["g"]
                            if gg % 4 == 0:
                                obatch[u["h"]] = outp.tile(
                                    [128, 4 * GQ, 128], F32, tag="ot", name="ot"
                                )
                            ob = obatch[u["h"]]
                            for il in range(GQ):
                                # normalize on ScalarE (Copy with per-
                                # partition scale): keeps the acc-bank
                                # recycle path off the congested DVE queue
                                nc.scalar.activation(
                                    out=ob[:, (gg % 4) * GQ + il, :],
                                    in_=gsf["accs"][il][:, 0:128],
                                    func=mybir.ActivationFunctionType.Copy,
                                    scale=rec[:, il, :],
                                )
                            if gg % 4 == 3:
                                r0 = (gg - 3) * GQ * 128
                                r1 = (gg + 1) * GQ * 128
                                hh = u["h"]

                                def out_dma(ob=ob, r0=r0, r1=r1, hh=hh):
                                    nc.sync.dma_start(
                                        out=o_d[
                                            r0:r1, hh * D : (hh + 1) * D
                                        ].rearrange("(t p) c -> p t c", p=128),
                                        in_=ob,
                                    )

                                if gg == NG - 1:
                                    # tail: nothing left to overlap with --
                                    # emit inline instead of post-flush
                                    out_dma()
                                else:
                                    # stage the store 2 units out so it
                                    # reaches the Sync queue head after
                                    # normalize is done
                                    at(u["idx"] + SKEW + 2, out_dma)

                    return f

                nb = len(u["blocks"])
                first_f = u["si"] == 0
                for bi, blk in enumerate(u["blocks"]):
                    chunks.append(
                        emit_block(blk, first_f and bi == 0, bi == nb - 1)
                    )
                return chunks

            def emit_qk_interleaved(u, w):
                """Emit u's QK matmuls with w's PV chunks woven between."""
                st2 = psum_st.tile([128, STRIP_W], F32, tag="st", name="st2")
                pt2 = pstage.tile([128, STRIP_W], BF16, tag="pt", name="pt2")
                u["st2"], u["pt2"] = st2, pt2
                chunks = pv_chunks(w) if w is not None else []
                ci = 0
                if u["has_diag"]:
                    nc.tensor.matmul(
                        st2[:, 0:DIAG_W],
                        lhsT=maskT,
                        rhs=iext,
                        start=True,
                        stop=True,
                    )
                g0 = u["g"] * GQ
                nq = len(u["blocks"])
                for qi, (j, c0, w_, so_b) in enumerate(u["blocks"]):
                    t0 = c0 // 128
                    nc.tensor.matmul(
                        st2[:, so_b : so_b + w_],
                        lhsT=kT[:, j, :],
                        rhs=qT[u["h"]][:, t0 : t0 + w_ // 128, :],
                        start=j < g0,
                        stop=True,
                    )
                    want = ((qi + 1) * len(chunks) + nq - 1) // nq
                    while ci < min(want, len(chunks)):
                        chunks[ci]()
                        ci += 1
                while ci < len(chunks):
                    chunks[ci]()
                    ci += 1

            def emit_exp(u, sidx):
                if sidx % DVE_MOD == DVE_PHASE:
                    nc.vector._custom_dve(
                        exp2_op,
                        out=u["pt2"][:, 0 : u["width"]].bitcast(I16),
                        in0=u["st2"][:, 0 : u["width"]],
                        in1=gconst[:, 0:1],
                        s0=EXP2_M,
                        s1=EXP2_A0,
                        imm2=EXP2_C,
                    )
                else:
                    nc.scalar.activation(
                        out=u["pt2"][:, 0 : u["width"]],
                        in_=u["st2"][:, 0 : u["width"]],
                        func=mybir.ActivationFunctionType.Exp,
                        scale=ACT_SCALE,
                    )

            pending = []
            for sidx, u in enumerate(units):
                w = None
                if len(pending) >= SKEW:
                    w = pending.pop(0)
                emit_qk_interleaved(u, w)
                emit_exp(u, sidx)
                for fn, args in sched[sidx]:
                    fn(*args)
                pending.append(u)
            for w in pending:
                for f in pv_chunks(w):
                    f()
            for fn, args in sched[nu]:
                fn(*args)

    nc.compile()
    return nc


_NC = None


def _get_nc():
    global _NC
    if _NC is None:
        _NC = _build()
    return _NC


def _shard(q, k, v):
    in_maps = []
    for c in range(NCORES):
        g = c // 2
        in_maps.append(
            {
                "q": np.ascontiguousarray(
                    q[:, c * H_PER_CORE * D : (c + 1) * H_PER_CORE * D],
                    dtype=np.float32,
                ),
                "k": np.ascontiguousarray(k[:, g * D : (g + 1) * D], dtype=np.float32),
                "v": np.ascontiguousarray(v[:, g * D : (g + 1) * D], dtype=np.float32),
            }
        )
    return in_maps


def _run(q, k, v, trace=False):
    nc = _get_nc()
    res = run_bass_kernel_spmd(
        nc, _shard(q, k, v), core_ids=list(range(NCORES)), trace=trace
    )
    out = np.concatenate(
        [np.asarray(res.results[c]["out"]) for c in range(NCORES)], axis=1
    )
    return out.astype(np.float32, copy=False), res


def kernel(q, k, v):
    out, _ = _run(np.asarray(q), np.asarray(k), np.asarray(v), trace=False)
    return out


# revision 42
# speedup vs baseline: 1.0391x; 1.0391x over previous
"""Trainium2 Bass kernel: causal GQA attention (prefill), 8-core tensor-parallel.

Problem: q [4096, 16*128], k/v [4096, 4*128], f32. 16 query heads, 4 kv heads,
head_dim 128, causal softmax(q k^T / sqrt(d)) v.

Sharding: head-parallel across 8 NeuronCores. Core c owns query heads
{2c, 2c+1}, which both belong to kv head c//2. Each core runs full causal
attention over its 2 heads; no cross-core communication.

Per-core kernel (N=4096 tokens, 32 token tiles of 128):
  - Inputs DMA'd in chunks into f32 SBUF staging. q/k tiles are cast to bf16
    (q on GpSimd, k on VectorE with the score pre-scale folded in) and
    transposed by the DMA xbar (dma_start_transpose, 4 tiles per dispatch,
    blocked 3D destination) into qT/kT [d=128, 4096] bf16 -- no TensorE or
    PSUM involvement in transposes at all. kT carries 128*log2(e)/sqrt(d)
    so scores arrive as u = 128*log2(exp(s/sqrt(d))). v is cast to bf16
    with a ones-column appended (vones) so the PV matmul's column 128
    accumulates the softmax denominator for free.
  - Scores computed transposed: S^T[m, qcols] = kT_j.T @ qT (PSUM f32) over
    [128, <=1024] strips (2 PSUM banks x 3 buffers).
  - Softmax exp is split across TWO engines: ScalarE activation
    (exp(u*ln2/128)) for most strips, and a custom Vector-engine DVE op
    (EXP2_SPLICE_ANT, registered at import) for every third strip: it
    computes the bf16 BIT PATTERN of 2^(u/128) directly -- magic-constant
    round to the 128-grid, |r| + r^2 correction polynomial spliced onto the
    exponent field, written as int16 and bitcast to bf16. Masked scores
    (-1e9) saturate the int16 convert to 0x8000 = -0.0. ~0.26% rms error on
    those strips, comparable to bf16 quantization.
  - pT strips are directly the stationary operand for PV:
    acc[q, 129] += pT_j.T @ [v_j | 1].
  - PE stream is software-pipelined with a TWO-strip skew and block-level
    interleave: strip i's PV matmuls (129 cols, shorter than their
    LDWEIGHTS) are emitted between strip i+2's QK matmuls (256+ cols) so
    every PV weight load hides under a QK matmul and no PV ever stalls the
    in-order PE queue waiting on exp.
  - Query groups are 2 tiles (256 cols); both accumulators share ONE PSUM
    bank (two banks, double buffered, so a group's PVs never wait on the
    previous group's normalize), zero-filled once per group by a lazy dummy
    matmul so all PV matmuls accumulate with start=False.
  - Causal: only blocks j<=i computed. The two diagonal blocks of a group
    are packed as one 512-col unit at strip offset 0, masked by a SINGLE
    start=True PE matmul (maskT.T @ [I|0|I|0]) writing [mask|0|mask|0];
    scores accumulate on top (start=False). No max-subtraction (scores ~
    N(0,1): exp cannot overflow).
  - Normalize: one batched reciprocal per group ([128, GQ]), then
    out[q, d] = acc[:, :128] * rec on VectorE, DMA to DRAM.
"""

import sys

for _p in ("/opt/trn_rl_repo",):
    if _p not in sys.path:
        sys.path.insert(0, _p)

import numpy as np

import concourse.bacc as bacc
import concourse.mybir as mybir
import concourse.tile as tile
from concourse.bass_utils import run_bass_kernel_spmd
from concourse.masks import make_identity

F32 = mybir.dt.float32
BF16 = mybir.dt.bfloat16
I16 = mybir.dt.int16

N = 4096
D = 128
H_PER_CORE = 2
NCORES = 8
NT = N // 128          # 32 token tiles
GQ = 2                 # q-tiles per group (256 query columns)
NG = NT // GQ          # 16 groups
SCALE = float(1.0 / np.sqrt(np.float32(D)))
MASK_VAL = -1e9
STRIP_W = 1024         # strip width in score cols (2 PSUM banks)
DIAG_W = 512           # padded diagonal unit width (1 PSUM bank)
SKEW = 2               # strips of PE software-pipeline skew

# scores arrive pre-scaled: u = s * PRESCALE with exp(s*SCALE) = 2^(u/128)
PRESCALE = float(np.float32(128.0 * np.log2(np.e) * SCALE))
ACT_SCALE = float(np.log(2.0) / 128.0)  # ScalarE: exp(u*ln2/128) = 2^(u/128)

# EXP2_SPLICE_ANT constants (see _register_exp2_op)
EXP2_M = float(np.float32(1.5 * 2**30))  # magic: snaps u to the 128 grid
EXP2_A0 = 16256.0                        # 127*128: bf16 exponent bias splice
EXP2_G = -0.3448068249203487             # |r| coefficient (kink term)
EXP2_C = 0.0027100904874714006           # r^2 coefficient
DVE_MOD = 3                              # strips with idx % MOD == PHASE -> DVE
DVE_PHASE = 1


def _exp2_ref(in0, in1, s0, s1, imm2):
    x = in0.astype(np.float32)
    t = (x + np.float32(s0)).astype(np.float32)
    n = (t - np.float32(s0)).astype(np.float32)
    ar = np.abs((x - n).astype(np.float32))
    return ((np.float32(imm2) * ar + in1) * ar + (x + np.float32(s1))).astype(
        np.float32
    )


def _register_exp2_op():
    """Register the custom DVE op computing int16 bf16-bits of 2^(Src0/128).

    out = ((C2*|r| + C3)*|r|) + (Src0 + C1),  r = Src0 - round128(Src0)
    with C0 = 1.5*2^30 (magic), C1 = 16256, C2 = r^2 coeff, C3 = |r| coeff
    (spilled to Src1 as a [P,1] broadcast). Output dtype int16: the f32->i16
    convert rounds and saturates, so masked scores (~-1e9) land at -0.0."""
    import concourse.dve_ops as dve_ops_mod
    from concourse.dve_ops import DveOp
    from concourse.dve_spec import (
        AluOp,
        Bin,
        C0,
        C1,
        C2,
        C3,
        Spec,
        Src0,
        _spill_c3_to_src1,
    )

    name = "EXP2_SPLICE_ANT"
    for o in dve_ops_mod.OPS:
        if o.name == name:
            return o
    dve_ops_mod._SUB_OPCODE_FOR_NAME[name] = 17

    t = Src0 + C0
    n = t - C0
    ar = Bin(AluOp.ABSOLUTE_DIFF, Src0, n)
    v1 = Src0 + C1
    u1 = ar * C2
    u2 = u1 + C3
    u3 = u2 * ar
    body = u3 + v1
    op = DveOp(
        name,
        Spec(body=_spill_c3_to_src1(body), reference=_exp2_ref),
        subdim=False,
        uops_sha={"v3": "f8e4d417c7e21f04", "v4": "78ec334198c67423"},
    )
    dve_ops_mod.OPS.append(op)
    return op


def _build():
    exp2_op = _register_exp2_op()
    nc = bacc.Bacc(
        "TRN2",
        target_bir_lowering=False,
        debug=False,
        enable_asserts=False,
        num_devices=NCORES,
    )
    q_d = nc.dram_tensor("q", [N, H_PER_CORE * D], F32, kind="ExternalInput").ap()
    k_d = nc.dram_tensor("k", [N, D], F32, kind="ExternalInput").ap()
    v_d = nc.dram_tensor("v", [N, D], F32, kind="ExternalInput").ap()
    o_d = nc.dram_tensor("out", [N, H_PER_CORE * D], F32, kind="ExternalOutput").ap()

    with tile.TileContext(nc) as tc:
        with (
            tc.tile_pool(name="consts", bufs=1) as consts,
            tc.tile_pool(name="big", bufs=1) as big,
            tc.tile_pool(name="cstage", bufs=4) as cstage,
            tc.tile_pool(name="pstage", bufs=8) as pstage,
            tc.tile_pool(name="outp", bufs=2) as outp,
            tc.tile_pool(name="rpool", bufs=8) as rpool,
            tc.tile_pool(name="pst", bufs=3, space="PSUM") as psum_st,
            tc.tile_pool(name="pacc", bufs=2, space="PSUM") as psum_acc,
        ):
            # diag mask, stored TRANSPOSED (maskT[q, m] = 0 if m <= q else
            # MASK_VAL) so a matmul maskT.T @ I writes mask[m, q] into PSUM;
            # the diagonal QK matmuls then accumulate scores on top.
            maskT = consts.tile([128, 128], BF16)
            nc.gpsimd.memset(maskT, 0.0)
            nc.gpsimd.affine_select(
                out=maskT,
                in_=maskT,
                compare_op=mybir.AluOpType.is_ge,
                fill=MASK_VAL,
                base=0,
                # keep 0 where (x=q) - (y=m) >= 0, i.e. m <= q
                pattern=[[-1, 128]],
                channel_multiplier=1,
            )
            zeros_bf = consts.tile([128, 128], BF16)
            nc.vector.memset(zeros_bf, 0.0)
            # [I | 0 | I | 0]: moving operand writing the whole 512-col
            # diagonal unit [mask | 0 | mask | 0] in ONE start=True matmul
            # (two start=True in the same bank would clear each other's
            # has_written bits).
            iext = consts.tile([128, DIAG_W], BF16)
            nc.vector.memset(iext, 0.0)
            make_identity(nc, iext[:, 0:128], nomemset=True)
            make_identity(nc, iext[:, 256:384], nomemset=True)
            # [P, 1] broadcast of the spilled |r| coefficient for the DVE op
            gconst = consts.tile([128, 1], F32)
            nc.vector.memset(gconst, EXP2_G)

            # f32 staging ([p, tile, col], p = token % 128). q is split per
            # head so h1's 2MB streams in during h0's compute instead of
            # competing with startup-critical h0/k/v loads.
            qst = [
                big.tile([128, NT, D], F32, tag=f"qst{h}", name=f"qst{h}")
                for h in range(H_PER_CORE)
            ]
            kst = big.tile([128, NT, D], F32, tag="kst")
            vst = big.tile([128, NT, D], F32, tag="vst")

            # [d, token-tile, token%128] -- 3D so DMA-transpose destinations
            # and matmul operands are natural tile slices
            qT = [
                big.tile([128, NT, 128], BF16, tag=f"qT{h}", name=f"qT{h}")
                for h in range(H_PER_CORE)
            ]
            kT = big.tile([128, NT, 128], BF16, tag="kT")
            vones = big.tile([128, NT, 129], BF16, tag="vones")

            # ---- input loads (4-tile chunks) ----
            # Loads, DMA-xbar transposes and output stores all share the
            # Sync queue and each dispatch costs ~0.5-0.7us of desc-gen, so
            # loads are interleaved with the prep stream rather than all
            # upfront (which would park the first transpose ~25us out).
            def dma_load_kv(t):
                nc.sync.dma_start(
                    out=kst[:, t : t + 4, :],
                    in_=k_d[t * 128 : (t + 4) * 128, :].rearrange(
                        "(t p) c -> p t c", p=128
                    ),
                )
                nc.sync.dma_start(
                    out=vst[:, t : t + 4, :],
                    in_=v_d[t * 128 : (t + 4) * 128, :].rearrange(
                        "(t p) c -> p t c", p=128
                    ),
                )

            def dma_load_q(h, t):
                nc.sync.dma_start(
                    out=qst[h][:, t : t + 4, :],
                    in_=q_d[
                        t * 128 : (t + 4) * 128, h * D : (h + 1) * D
                    ].rearrange("(t p) c -> p t c", p=128),
                )

            # Prep stages for one 4-tile batch. Casts park their tile in
            # `cst` until the (separately scheduled) transpose dispatch.
            cst = {}

            def prep_loads(t):
                dma_load_kv(t)
                dma_load_q(0, t)
                dma_load_q(1, t)

            def prep_casts(t, fast=False):
                ck = cstage.tile([128, 4, 128], BF16, tag="ck", name="ck")
                # fold the score pre-scale into the k cast; ScalarE keeps it
                # out of the DVE queue (whose exp strips feed PV tightly)
                nc.scalar.mul(ck, kst[:, t : t + 4, :], PRESCALE)
                cst[("k", t)] = ck
                for h in range(H_PER_CORE):
                    cq = cstage.tile(
                        [128, 4, 128], BF16, tag=f"cq{h}", name=f"cq{h}"
                    )
                    if fast:
                        # startup-critical: VectorE cast (~0.6us) instead of
                        # the slow GpSimd ucode (~1.9us)
                        nc.vector.tensor_copy(cq, qst[h][:, t : t + 4, :])
                    else:
                        nc.gpsimd.tensor_copy(cq, qst[h][:, t : t + 4, :])
                    cst[("q", h, t)] = cq
                nc.gpsimd.tensor_copy(vones[:, t : t + 4, 0:128], vst[:, t : t + 4, :])
                nc.gpsimd.memset(vones[:, t : t + 4, 128:129], 1.0)

            def prep_tps(t):
                # by the time these reach the queue head their casts are
                # long done, so they never stall the Sync queue
                nc.sync.dma_start_transpose(
                    out=kT[:, t : t + 4, :], in_=cst.pop(("k", t))[:, :, :]
                )
                for h in range(H_PER_CORE):
                    nc.sync.dma_start_transpose(
                        out=qT[h][:, t : t + 4, :],
                        in_=cst.pop(("q", h, t))[:, :, :],
                    )

            # upfront: loads for tiles 0..15, casts+transposes for 0..3 (the
            # rest are stage-scheduled between strip units)
            prep_loads(0)
            prep_casts(0, fast=True)
            prep_loads(4)
            prep_tps(0)
            prep_loads(8)
            prep_loads(12)

            # ---- build the full strip schedule ----
            # HEADS ARE INTERLEAVED (h0 g, h1 g, h0 g+1, ...): early groups
            # are tiny (1-2 strips) and burn through new k/q tiles faster
            # than the load+cast+transpose pipeline can deliver them; with
            # both heads consuming each tile the demand rate halves.
            # Strip 0 of each group leads with the 512-col diagonal unit
            # (blocks j=2g w256 at 0, j=2g+1 w128 at 256, pad 384..512)
            # followed by up to 2 off-diagonal blocks; remaining
            # off-diagonals pack 4 per strip. Every block start stays
            # 256-aligned inside 512-col PSUM banks.
            units = []  # flat list over groups/heads/strips
            for g in range(NG):
                for h in range(H_PER_CORE):
                    qc0 = g * GQ * 128
                    offd = [(j, qc0, GQ * 128) for j in range(g * GQ)]
                    strips = []
                    first_blocks = [
                        (g * GQ, qc0, 256, 0),
                        (g * GQ + 1, qc0 + 128, 128, 256),
                    ]
                    so = DIAG_W
                    while offd and so < STRIP_W:
                        j, c0, w = offd.pop(0)
                        first_blocks.append((j, c0, w, so))
                        so += w
                    strips.append((first_blocks, so, True))
                    while offd:
                        blocks = []
                        so = 0
                        while offd and so < STRIP_W:
                            j, c0, w = offd.pop(0)
                            blocks.append((j, c0, w, so))
                            so += w
                        strips.append((blocks, so, False))

                    # last-emitted PV per accumulator gets the stop flag
                    last_pv = {}
                    for si, (blocks, _, _) in enumerate(strips):
                        for j, c0, w, so_b in blocks:
                            for il in range(GQ):
                                if g * GQ + il >= j:
                                    last_pv[il] = (si, j)

                    ns = len(strips)
                    for si, (blocks, width, has_diag) in enumerate(strips):
                        units.append(
                            dict(
                                h=h,
                                g=g,
                                si=si,
                                idx=len(units),
                                blocks=blocks,
                                width=width,
                                has_diag=has_diag,
                                last=(si == ns - 1),
                                last_pv=last_pv,
                            )
                        )

            # ---- stage-scheduled prep/IO emission ----
            # Each prep stage (load -> cast -> transpose) is emitted a few
            # units after its upstream stage, so by the time an instruction
            # reaches the head of its in-order queue its dependencies are
            # complete and it never parks the queue. All batches land well
            # before first use (tiles 4t first read by group pair 2t).
            pg = {}
            for idx, u in enumerate(units):
                pg.setdefault(u["g"], idx)
            nu = len(units)
            sched = [[] for _ in range(nu + 1)]

            def at(idx, fn, *args):
                sched[min(max(idx, 0), nu)].append((fn, args))

            at(1, prep_casts, 4, True)
            at(3, prep_tps, 4)
            at(3, prep_casts, 8)
            at(5, prep_tps, 8)
            at(5, prep_casts, 12)
            at(7, prep_tps, 12)
            for t in range(16, NT, 4):
                gA = (t - 12) // 2
                i0 = pg[gA]
                at(i0, prep_loads, t)
                at(i0 + 2, prep_casts, t)
                at(i0 + 4, prep_tps, t)

            # ---- software-pipelined emission, two-strip skew ----
            # The PE queue is in-order: a strip's PV matmuls would stall the
            # queue waiting on that strip's exp. Interleave strip i's PV
            # matmuls BETWEEN strip i+2's QK matmuls: by then exp(i) is long
            # done, and each 129-col PV weight load hides under a 256-col QK
            # matmul. Group normalize rides in the PV stream.
            gstate = {}  # (h, g) -> dict(acc2, accs)
            obatch = {}  # h -> current 4-group output batch tile

            def pv_chunks(u):
                """Per-block PV emitters for strip u (plus group bookkeeping)."""
                chunks = []
                g0 = u["g"] * GQ
                key = (u["h"], u["g"])

                def emit_block(blk, first, last):
                    def f():
                        gs = gstate.get(key)
                        if gs is None and first:
                            # both q-tile accumulators in ONE psum bank; a
                            # single dummy matmul (zeros stationary,
                            # start=True) clears has_written for the whole
                            # bank so every PV accumulates with start=False.
                            acc2 = psum_acc.tile(
                                [128, GQ, 129], F32, tag="acc", name="acc2"
                            )
                            gs = gstate[key] = dict(
                                acc2=acc2, accs=[acc2[:, a, :] for a in range(GQ)]
                            )
                            nc.tensor.matmul(
                                acc2[:, 0, 0:1],
                                lhsT=zeros_bf,
                                rhs=iext[:, 128:129],
                                start=True,
                                stop=True,
                            )
                        gs = gstate[key]
                        j, c0, w, so_b = blk
                        pt2 = u["pt2"]
                        for il in range(GQ):
                            i = g0 + il
                            if i < j:
                                continue
                            off = so_b + i * 128 - c0
                            nc.tensor.matmul(
                                gs["accs"][il],
                                lhsT=pt2[:, off : off + 128],
                                rhs=vones[:, j, :],
                                start=False,
                                stop=(u["last_pv"][il] == (u["si"], j)),
                            )
                        if last and u["last"]:
                            gsf = gstate.pop(key)
                            rec = rpool.tile(
                                [128, GQ, 1], F32, tag="rec", name="rec"
                            )
                            nc.vector.reciprocal(rec, gsf["acc2"][:, :, 128:129])
                            # normalized outputs collect in an 8-tile batch
                            # buffer; ONE output DMA per 4 groups (each
                            # dispatch costs ~0.5us of Sync desc-gen)
                            gg = u["g"]
                            if gg % 4 == 0:
                                obatch[u["h"]] = outp.tile(
                                    [128, 4 * GQ, 128], F32, tag="ot", name="ot"
                                )
                            ob = obatch[u["h"]]
                            for il in range(GQ):
                                # normalize on ScalarE (Copy with per-
                                # partition scale): keeps the acc-bank
                                # recycle path off the congested DVE queue
                                nc.scalar.activation(
                                    out=ob[:, (gg % 4) * GQ + il, :],
                                    in_=gsf["accs"][il][:, 0:128],
                                    func=mybir.ActivationFunctionType.Copy,
                                    scale=rec[:, il, :],
                                )
                            if gg % 4 == 3:
                                r0 = (gg - 3) * GQ * 128
                                r1 = (gg + 1) * GQ * 128
                                hh = u["h"]

                                def out_dma(ob=ob, r0=r0, r1=r1, hh=hh):
                                    nc.sync.dma_start(
                                        out=o_d[
                                            r0:r1, hh * D : (hh + 1) * D
                                        ].rearrange("(t p) c -> p t c", p=128),
                                        in_=ob,
                                    )

                                # stage the store 2 units out so it reaches
                                # the Sync queue head after normalize is done
                                at(u["idx"] + SKEW + 2, out_dma)

                    return f

                nb = len(u["blocks"])
                first_f = u["si"] == 0
                for bi, blk in enumerate(u["blocks"]):
                    chunks.append(
                        emit_block(blk, first_f and bi == 0, bi == nb - 1)
                    )
                return chunks

            def emit_qk_interleaved(u, w):
                """Emit u's QK matmuls with w's PV chunks woven between."""
                st2 = psum_st.tile([128, STRIP_W], F32, tag="st", name="st2")
                pt2 = pstage.tile([128, STRIP_W], BF16, tag="pt", name="pt2")
                u["st2"], u["pt2"] = st2, pt2
                chunks = pv_chunks(w) if w is not None else []
                ci = 0
                if u["has_diag"]:
                    nc.tensor.matmul(
                        st2[:, 0:DIAG_W],
                        lhsT=maskT,
                        rhs=iext,
                        start=True,
                        stop=True,
                    )
                g0 = u["g"] * GQ
                nq = len(u["blocks"])
                for qi, (j, c0, w_, so_b) in enumerate(u["blocks"]):
                    t0 = c0 // 128
                    nc.tensor.matmul(
                        st2[:, so_b : so_b + w_],
                        lhsT=kT[:, j, :],
                        rhs=qT[u["h"]][:, t0 : t0 + w_ // 128, :],
                        start=j < g0,
                        stop=True,
                    )
                    want = ((qi + 1) * len(chunks) + nq - 1) // nq
                    while ci < min(want, len(chunks)):
                        chunks[ci]()
                        ci += 1
                while ci < len(chunks):
                    chunks[ci]()
                    ci += 1

            def emit_exp(u, sidx):
                if sidx % DVE_MOD == DVE_PHASE:
                    nc.vector._custom_dve(
                        exp2_op,
                        out=u["pt2"][:, 0 : u["width"]].bitcast(I16),
                        in0=u["st2"][:, 0 : u["width"]],
                        in1=gconst[:, 0:1],
                        s0=EXP2_M,
                        s1=EXP2_A0,
                        imm2=EXP2_C,
                    )
                else:
                    nc.scalar.activation(
                        out=u["pt2"][:, 0 : u["width"]],
                        in_=u["st2"][:, 0 : u["width"]],
                        func=mybir.ActivationFunctionType.Exp,
                        scale=ACT_SCALE,
                    )

            pending = []
            for sidx, u in enumerate(units):
                w = None
                if len(pending) >= SKEW:
                    w = pending.pop(0)
                emit_qk_interleaved(u, w)
                emit_exp(u, sidx)
                for fn, args in sched[sidx]:
                    fn(*args)
                pending.append(u)
            for w in pending:
                for f in pv_chunks(w):
                    f()
            for fn, args in sched[nu]:
                fn(*args)

    nc.compile()
    return nc


_NC = None


def _get_nc():
    global _NC
    if _NC is None:
        _NC = _build()
    return _NC


def _shard(q, k, v):
    in_maps = []
    for c in range(NCORES):
        g = c // 2
        in_maps.append(
            {
                "q": np.ascontiguousarray(
                    q[:, c * H_PER_CORE * D : (c + 1) * H_PER_CORE * D],
                    dtype=np.float32,
                ),
                "k": np.ascontiguousarray(k[:, g * D : (g + 1) * D], dtype=np.float32),
                "v": np.ascontiguousarray(v[:, g * D : (g + 1) * D], dtype=np.float32),
            }
        )
    return in_maps


def _run(q, k, v, trace=False):
    nc = _get_nc()
    res = run_bass_kernel_spmd(
        nc, _shard(q, k, v), core_ids=list(range(NCORES)), trace=trace
    )
    out = np.concatenate(
        [np.asarray(res.results[c]["out"]) for c in range(NCORES)], axis=1
    )
    return out.astype(np.float32, copy=False), res


def kernel(q, k, v):
    out, _ = _run(np.asarray(q), np.asarray(k), np.asarray(v), trace=False)
    return out
